# revision 25
# baseline (speedup 1.0000x reference)
"""AttentionBlock (GroupNorm + 4-head self-attention + proj + residual) on 8 TRN2 cores.

Data-parallel over batch: 16 batch elements -> 2 per NeuronCore. All matmuls run
in float32r (full-rate fp32 mode, ~1.2e-4 relative rounding).

Layout strategy (per core, per batch element; channels C=512, tokens N=1024):
  - GroupNorm stats via bn_stats per 128-channel chunk + DMA regroup to
    [32 groups, 16ch*stats] + bn_aggr; per-channel scale/bias folded into one
    tensor_scalar pass that also rounds to f32r.
  - QKV: lhsT = host-pretransposed qkv_w^T chunks; Q,K produced as [d, n]
    (channel-major), V produced directly transposed as V^T = h^T @ Wv^T [n, d].
  - Attention per (head, query-half): S^T = K^T Q via lhsT=K-slice (no PE
    transposes anywhere); exp on ACT straight out of PSUM (softmax max-shift
    skipped: inputs are bounded N(0,1)-scale so exp cannot overflow);
    denominator = ones^T @ P via PE; normalization: r = 1/denom broadcast
    across partitions with a K=1 outer-product matmul, applied in the
    PSUM->SBUF copy of O^T.
  - proj: lhsT = proj_w^T chunks over O^T; residual + bias fused in one
    scalar_tensor_tensor. Output fp32.
"""

import math

import numpy as np

import concourse.bass as bass
import concourse.mybir as mybir
from concourse import tile
from concourse.bass_utils import run_bass_kernel_spmd

# problem constants (self-contained by contract)
B, C, H, W = 16, 512, 32, 32
N = H * W
HEADS, D = 4, 128
G = 32
EPS = 1e-5
SCALE = 1.0 / math.sqrt(D)
NCORES = 8
NB = B // NCORES  # batch elems per core
CK = C // 128     # channel chunks
NT = N // 128     # token tiles
F32 = mybir.dt.float32
F32R = mybir.dt.float32r
F16 = mybir.dt.float16
FX = mybir.ActivationFunctionType
ALU = mybir.AluOpType

WARM_MMS = 44  # PE warmup matmuls issued during the groupnorm prologue


# --- workaround: this walrus encodes at most ONE sync wait per instruction ---
_waitctr = [0]


def _split_multiwait(nc):
    for fn in nc.m.functions:
        for bb in fn.blocks:
            out = []
            changed = False
            for inst in bb.instructions:
                si = inst.sync_info
                if si is not None and len(si.on_wait) > 1:
                    waits = list(si.on_wait)
                    for wt in waits[:-1]:
                        _waitctr[0] += 1
                        nop = mybir.InstNoOp(
                            name=f"I-waitsplit-{_waitctr[0]}", ins=[], outs=[]
                        )
                        nop.engine = inst.engine
                        nop.sync_info = mybir.SyncInfo(on_wait=[wt], on_update=[])
                        out.append(nop)
                    inst.sync_info = mybir.SyncInfo(
                        on_wait=[waits[-1]], on_update=list(si.on_update)
                    )
                    changed = True
                out.append(inst)
            if changed:
                bb.instructions = out


def _build(loop_n=None, ablate=()):
    ab = set(ablate)
    nc = bass.Bass("TRN2", target_bir_lowering=False, debug=False)

    x_d = nc.dram_tensor("x", [NB, C, N], F32, kind="ExternalInput")
    qwT_d = nc.dram_tensor("qwT", [C, 3 * C], F16, kind="ExternalInput")
    qb_d = nc.dram_tensor("qb", [3 * C], F32, kind="ExternalInput")
    qb16_d = nc.dram_tensor("qb16", [3 * C], F16, kind="ExternalInput")
    pwT_d = nc.dram_tensor("pwT", [C, C], F16, kind="ExternalInput")
    pb_d = nc.dram_tensor("pb", [C], F32, kind="ExternalInput")
    nw_d = nc.dram_tensor("nw", [C], F32, kind="ExternalInput")
    nb_d = nc.dram_tensor("nb", [C], F32, kind="ExternalInput")
    y_d = nc.dram_tensor("y", [NB, C, N], F32, kind="ExternalOutput")

    with tile.TileContext(nc) as tc:
        import contextlib

        with contextlib.ExitStack() as ctx:
            wpool = ctx.enter_context(tc.tile_pool(name="wpool", bufs=1))
            xpool = ctx.enter_context(tc.tile_pool(name="xpool", bufs=4))
            x2pool = ctx.enter_context(tc.tile_pool(name="x2pool", bufs=2))
            spool = ctx.enter_context(tc.tile_pool(name="spool", bufs=2))
            stp = ctx.enter_context(tc.tile_pool(name="stp", bufs=5))
            hpool = ctx.enter_context(tc.tile_pool(name="hpool", bufs=5))
            qkpool = ctx.enter_context(tc.tile_pool(name="qkpool", bufs=8))
            vpool = ctx.enter_context(tc.tile_pool(name="vpool", bufs=9))
            ppool = ctx.enter_context(tc.tile_pool(name="ppool", bufs=16))
            opool = ctx.enter_context(tc.tile_pool(name="opool", bufs=5))
            ounpool = ctx.enter_context(tc.tile_pool(name="ounpool", bufs=10))
            ypool = ctx.enter_context(tc.tile_pool(name="ypool", bufs=2))
            ps_s = ctx.enter_context(tc.tile_pool(name="ps_s", bufs=2, space="PSUM"))
            ps_d = ctx.enter_context(tc.tile_pool(name="ps_d", bufs=4, space="PSUM"))
            ps_q = ctx.enter_context(tc.tile_pool(name="ps_q", bufs=2, space="PSUM"))

            # ---- constants & weights ----
            ones_col = wpool.tile([128, 1], F16, tag="ones_col")
            nc.gpsimd.memset(ones_col[:], 1.0)
            ones_row = wpool.tile([1, 128], F16, tag="ones_row")
            nc.gpsimd.memset(ones_row[:], 1.0)
            ones_w = wpool.tile([128, 512], F16, tag="ones_w")
            nc.gpsimd.memset(ones_w[:], 1.0)

            # PE warmup: overlaps the DMA/stats prologue so HAM reaches 2.4GHz
            # before the qkv matmuls start.
            warm_ps = ps_q.tile([128, 512], F32, tag="psq")
            for _ in range(0 if "no_warm" in ab else WARM_MMS):
                nc.tensor.matmul(
                    warm_ps[0:1, :], ones_col[:], ones_w[:], start=True, stop=True
                )

            wq = []
            for ck in range(CK):
                t = wpool.tile([128, 3 * C], F16, tag=f"wq{ck}")
                nc.sync.dma_start(
                    t[:], qwT_d.ap()[128 * ck : 128 * (ck + 1), :]
                )
                wq.append(t)
            wp = []
            for ck in range(CK):
                t = wpool.tile([128, C], F16, tag=f"wp{ck}")
                nc.sync.dma_start(
                    t[:], pwT_d.ap()[128 * ck : 128 * (ck + 1), :]
                )
                wp.append(t)
            qbv = []
            for t8 in range(8):
                t = wpool.tile([128, 1], F32, tag=f"qbv{t8}")
                nc.sync.dma_start(
                    t[:],
                    qb_d.ap()[128 * t8 : 128 * (t8 + 1)].rearrange(
                        "(p o) -> p o", o=1
                    ),
                )
                qbv.append(t)
            qbv_row = wpool.tile([1, C], F16, tag="qbv_row")
            nc.sync.dma_start(
                qbv_row[:],
                qb16_d.ap()[2 * C : 3 * C].rearrange("(o n) -> o n", o=1),
            )
            pbv = []
            for t4 in range(CK):
                t = wpool.tile([128, 1], F32, tag=f"pbv{t4}")
                nc.sync.dma_start(
                    t[:],
                    pb_d.ap()[128 * t4 : 128 * (t4 + 1)].rearrange(
                        "(p o) -> p o", o=1
                    ),
                )
                pbv.append(t)
            eps128 = wpool.tile([128, 1], F32, tag="eps")
            nc.gpsimd.memset(eps128[:], EPS)
            nwv, nbv = [], []
            for ck in range(CK):
                t = wpool.tile([128, 1], F32, tag=f"nwv{ck}")
                nc.sync.dma_start(
                    t[:],
                    nw_d.ap()[128 * ck : 128 * (ck + 1)].rearrange("(p o) -> p o", o=1),
                )
                nwv.append(t)
                t2 = wpool.tile([128, 1], F32, tag=f"nbv{ck}")
                nc.sync.dma_start(
                    t2[:],
                    nb_d.ap()[128 * ck : 128 * (ck + 1)].rearrange("(p o) -> p o", o=1),
                )
                nbv.append(t2)

            # ---- per-batch phase emitters ----
            def emit_norm(b):
                """GroupNorm via per-channel bn_stats + XOR-butterfly partition
                all-reduce (stream_shuffle) within each 16-channel group."""
                hs = []
                for ck in range(CK):
                    xt = xpool.tile([128, N], F32, tag="x", name=f"xt{b}_{ck}")
                    nc.sync.dma_start(
                        xt[:], x_d.ap()[b, 128 * ck : 128 * (ck + 1), :]
                    )
                    st = stp.tile([128, 12], F32, tag="st")
                    nc.vector.bn_stats(st[:, 0:6], xt[:, 0:512])
                    nc.vector.bn_stats(st[:, 6:12], xt[:, 512:1024])
                    str_ = st[:, :].rearrange("p (f t) -> p t f", t=3)
                    sum_m = stp.tile([128, 1], F32, tag="sum_m")
                    nc.vector.tensor_reduce(
                        sum_m[:], str_[:, 1, :], mybir.AxisListType.X, ALU.add
                    )
                    msq = stp.tile([128, 4], F32, tag="msq")
                    nc.vector.tensor_mul(msq[:], str_[:, 1, :], str_[:, 1, :])
                    sum_m2 = stp.tile([128, 1], F32, tag="sum_m2")
                    nc.vector.tensor_reduce(
                        sum_m2[:], msq[:], mybir.AxisListType.X, ALU.add
                    )
                    sum_cv = stp.tile([128, 1], F32, tag="sum_cv")
                    nc.vector.tensor_reduce(
                        sum_cv[:], str_[:, 2, :], mybir.AxisListType.X, ALU.add
                    )
                    # sg = [sum(x), sum(x^2)] per channel (512 elems each)
                    sg = stp.tile([128, 2], F32, tag="sg")
                    nc.vector.tensor_scalar(
                        sg[:, 0:1], sum_m[:], 256.0, None, ALU.mult
                    )
                    nc.vector.scalar_tensor_tensor(
                        sg[:, 1:2], sum_m2[:], 256.0, sum_cv[:], ALU.mult, ALU.add
                    )
                    for k in (1, 2, 4, 8):
                        tmp = stp.tile([128, 2], F32, tag="shuf")
                        nc.vector.stream_shuffle(
                            tmp[:], sg[:], [i ^ k for i in range(32)]
                        )
                        sg2 = stp.tile([128, 2], F32, tag="sg", name=f"sg{b}_{ck}_{k}")
                        nc.vector.tensor_add(sg2[:], sg[:], tmp[:])
                        sg = sg2
                    mean = stp.tile([128, 1], F32, tag="mean")
                    nc.vector.tensor_scalar(
                        mean[:], sg[:, 0:1], 1.0 / 16384.0, None, ALU.mult
                    )
                    e2 = stp.tile([128, 1], F32, tag="e2")
                    nc.vector.tensor_scalar(
                        e2[:], sg[:, 1:2], 1.0 / 16384.0, None, ALU.mult
                    )
                    msq2 = stp.tile([128, 1], F32, tag="msq2")
                    nc.vector.tensor_scalar(
                        msq2[:], mean[:], mean[:, 0:1], None, ALU.mult
                    )
                    var = stp.tile([128, 1], F32, tag="var")
                    nc.vector.tensor_sub(var[:], e2[:], msq2[:])
                    std = stp.tile([128, 1], F32, tag="std")
                    nc.scalar.activation(
                        std[:], var[:], FX.Sqrt, bias=eps128[:, 0:1]
                    )
                    rstd = stp.tile([128, 1], F32, tag="rstd")
                    nc.vector.reciprocal(rstd[:], std[:])
                    svec = stp.tile([128, 1], F32, tag="svec")
                    nc.vector.tensor_mul(svec[:], nwv[ck][:], rstd[:])
                    tms = stp.tile([128, 1], F32, tag="tms")
                    nc.vector.tensor_scalar(
                        tms[:], mean[:], svec[:, 0:1], None, ALU.mult
                    )
                    tvec = stp.tile([128, 1], F32, tag="tvec")
                    nc.vector.tensor_sub(tvec[:], nbv[ck][:], tms[:])
                    ht = hpool.tile([128, N], F16, tag="h")
                    if "no_norm" in ab:
                        nc.vector.tensor_copy(ht[:], xt[:])
                    else:
                        nc.vector.tensor_scalar(
                            ht[:], xt[:], svec[:, 0:1], tvec[:, 0:1],
                            ALU.mult, ALU.add,
                        )
                    hs.append(ht)
                return hs

            def emit_qkv(b, hs):
                """QK as [d,n] per head-slice tile; V^T as [n, 512]."""
                qk = []
                for t8 in range(8):
                    sb = qkpool.tile([128, N], F16, tag="qk")
                    for qh in range(2):
                        ps = ps_q.tile([128, 512], F32, tag="psq", name=f"qkps{b}_{t8}_{qh}")
                        for ck in range(CK):
                            nc.tensor.matmul(
                                ps[:],
                                wq[ck][:, 128 * t8 : 128 * (t8 + 1)],
                                hs[ck][:, 512 * qh : 512 * (qh + 1)],
                                start=(ck == 0),
                                stop=(ck == CK - 1),
                            )
                        nc.vector.tensor_scalar(
                            sb[:, 512 * qh : 512 * (qh + 1)], ps[:],
                            qbv[t8][:, 0:1], None, ALU.add
                        )
                    qk.append(sb)
                vts = []
                for nt in range(NT):
                    ps = ps_q.tile([128, 512], F32, tag="psq", name=f"vps{b}_{nt}")
                    for ck in range(CK):
                        nc.tensor.matmul(
                            ps[:],
                            hs[ck][:, 128 * nt : 128 * (nt + 1)],
                            wq[ck][:, 2 * C : 3 * C],
                            start=(ck == 0),
                            stop=False,
                        )
                    nc.tensor.matmul(
                        ps[:], ones_row[:], qbv_row[:], start=False, stop=True
                    )
                    vt = vpool.tile([128, C], F16, tag="v")
                    nc.vector.tensor_copy(vt[:], ps[:])
                    vts.append(vt)
                return qk, vts

            # attention: units are (head, query-half); dn/O of unit u-1 are
            # interleaved with the S/exp stream of unit u to keep PE dense.
            def emit_unit(b, qk, vts, unit, prev):
                h, qh = unit
                q_sb, k_sb = qk[h], qk[HEADS + h]
                ptiles = []
                dn_ps = ot_ps = None
                if prev is not None:
                    if "no_dn" not in ab:
                        dn_ps = ps_d.tile([128, 512], F32, tag="psd")
                    ot_ps = ps_d.tile([128, 512], F32, tag="psd")
                for kt in range(NT):
                    if prev is not None:
                        ph, pptiles = prev[1][0], prev[2]
                        if "no_dn" not in ab:
                            nc.tensor.matmul(
                                dn_ps[0:1, :], ones_col[:], pptiles[kt][:],
                                start=(kt == 0), stop=(kt == NT - 1),
                            )
                        nc.tensor.matmul(
                            ot_ps[:], vts[kt][:, 128 * ph : 128 * (ph + 1)],
                            pptiles[kt][:],
                            start=(kt == 0), stop=(kt == NT - 1),
                        )
                    s_ps = ps_s.tile([128, 512], F32, tag="pss")
                    nc.tensor.matmul(
                        s_ps[:],
                        k_sb[:, 128 * kt : 128 * (kt + 1)],
                        q_sb[:, 512 * qh : 512 * (qh + 1)],
                        start=True, stop=True,
                    )
                    pt = ppool.tile([128, 512], F16, tag="p")
                    if "exp_dve" in ab:
                        nc.vector.tensor_copy(pt[:], s_ps[:])
                    else:
                        nc.scalar.activation(pt[:], s_ps[:], FX.Exp, scale=SCALE)
                    ptiles.append(pt)
                return ptiles, dn_ps, ot_ps

            def emit_unit_tail(b, prev, dn_ps, ot_ps, osb):
                """normalize: r = 1/denom; R = ones x r; o = O * R (from PSUM)."""
                h, qh = prev[1]
                if "no_dn" in ab:
                    nc.vector.tensor_copy(
                        osb[h][:, 512 * qh : 512 * (qh + 1)], ot_ps[:]
                    )
                    return
                r1 = spool.tile([1, 512], F32, tag="r1", bufs=4)
                nc.vector.reciprocal(r1[:], dn_ps[0:1, :])
                r1r = spool.tile([1, 512], F16, tag="r1r", bufs=4)
                nc.vector.tensor_copy(r1r[:], r1[:])
                R_ps = ps_q.tile([128, 512], F32, tag="psq")
                nc.tensor.matmul(R_ps[:], ones_row[:], r1r[:], start=True, stop=True)
                R_sb = spool.tile([128, 512], F32, tag="Rsb")
                nc.scalar.copy(R_sb[:], R_ps[:])
                nc.vector.tensor_mul(
                    osb[h][:, 512 * qh : 512 * (qh + 1)], ot_ps[:], R_sb[:]
                )

            def emit_attn(b, qk, vts, tail_ctx):
                osb = [
                    opool.tile([128, N], F16, tag="o", name=f"osb{b}_{i}")
                    for i in range(HEADS)
                ]
                units = [(h, qh) for h in range(HEADS) for qh in range(2)]
                prev = tail_ctx.get("prev")
                pending = tail_ctx.setdefault("pending", [])
                for u in units:
                    pt, dn_ps, ot_ps = emit_unit(b, qk, vts, u, prev)
                    if prev is not None:
                        pending.append((prev[0], prev, dn_ps, ot_ps, prev[3]))
                    while len(pending) > 1:
                        emit_unit_tail(*pending.pop(0))
                    prev = (b, u, pt, osb)
                tail_ctx["prev"] = prev
                return osb

            def emit_attn_flush(tail_ctx, vts_by_b):
                prev = tail_ctx.pop("prev", None)
                pending = tail_ctx.setdefault("pending", [])
                if prev is not None:
                    pb_, (ph, pqh), pptiles, posb = prev
                    dn_ps = None
                    if "no_dn" not in ab:
                        dn_ps = ps_d.tile([128, 512], F32, tag="psd")
                    ot_ps = ps_d.tile([128, 512], F32, tag="psd")
                    for kt in range(NT):
                        if "no_dn" not in ab:
                            nc.tensor.matmul(
                                dn_ps[0:1, :], ones_col[:], pptiles[kt][:],
                                start=(kt == 0), stop=(kt == NT - 1),
                            )
                        nc.tensor.matmul(
                            ot_ps[:], vts_by_b[pb_][kt][:, 128 * ph : 128 * (ph + 1)],
                            pptiles[kt][:],
                            start=(kt == 0), stop=(kt == NT - 1),
                        )
                    pending.append((pb_, prev, dn_ps, ot_ps, posb))
                while pending:
                    emit_unit_tail(*pending.pop(0))

            def emit_proj(b, osb):
                for t4 in range(CK):
                    xt2 = x2pool.tile([128, N], F32, tag="x2")
                    nc.sync.dma_start(
                        xt2[:], x_d.ap()[b, 128 * t4 : 128 * (t4 + 1), :]
                    )
                    yt = ypool.tile([128, N], F32, tag="y")
                    for qh in range(2):
                        ps = ps_q.tile([128, 512], F32, tag="psq", name=f"prps{b}_{t4}_{qh}")
                        for ck in range(CK):
                            nc.tensor.matmul(
                                ps[:],
                                wp[ck][:, 128 * t4 : 128 * (t4 + 1)],
                                osb[ck][:, 512 * qh : 512 * (qh + 1)],
                                start=(ck == 0),
                                stop=(ck == CK - 1),
                            )
                        nc.vector.scalar_tensor_tensor(
                            yt[:, 512 * qh : 512 * (qh + 1)], ps[:],
                            pbv[t4][:, 0:1], xt2[:, 512 * qh : 512 * (qh + 1)],
                            ALU.add, ALU.add,
                        )
                    nc.sync.dma_start(
                        y_d.ap()[b, 128 * t4 : 128 * (t4 + 1), :], yt[:]
                    )

            # ---- schedule ----
            def schedule():
                tail_ctx = {}
                vts_by_b = {}
                hs0 = emit_norm(0)
                qk0, vts0 = emit_qkv(0, hs0)
                vts_by_b[0] = vts0
                hs1 = emit_norm(1)
                osb0 = emit_attn(0, qk0, vts0, tail_ctx)
                qk1, vts1 = emit_qkv(1, hs1)
                vts_by_b[1] = vts1
                emit_attn_flush(tail_ctx, vts_by_b)
                emit_proj(0, osb0)
                osb1 = emit_attn(1, qk1, vts1, tail_ctx)
                emit_attn_flush(tail_ctx, vts_by_b)
                emit_proj(1, osb1)

            if loop_n is None:
                schedule()
            else:
                with tc.For_i(0, loop_n, 1):
                    schedule()

    _split_multiwait(nc)
    return nc


_CACHE = {}


def _get_program(loop_n=None, ablate=()):
    key = ("nc", loop_n, tuple(sorted(ablate)))
    if key not in _CACHE:
        _CACHE[key] = _build(loop_n, ablate)
    return _CACHE[key]


def _make_in_maps(inputs):
    x = np.ascontiguousarray(np.asarray(inputs["x"], dtype=np.float32))
    qkv_w = np.asarray(inputs["qkv_w"], dtype=np.float32)
    qkv_b = np.ascontiguousarray(np.asarray(inputs["qkv_b"], dtype=np.float32))
    proj_w = np.asarray(inputs["proj_w"], dtype=np.float32)
    proj_b = np.ascontiguousarray(np.asarray(inputs["proj_b"], dtype=np.float32))
    norm_w = np.ascontiguousarray(np.asarray(inputs["norm_w"], dtype=np.float32))
    norm_b = np.ascontiguousarray(np.asarray(inputs["norm_b"], dtype=np.float32))
    qwT = np.ascontiguousarray(qkv_w.T.astype(np.float16))
    pwT = np.ascontiguousarray(proj_w.T.astype(np.float16))
    qb16 = qkv_b.astype(np.float16)
    xs = x.reshape(NCORES, NB, C, N)
    in_maps = []
    for i in range(NCORES):
        in_maps.append(
            {
                "x": np.ascontiguousarray(xs[i]),
                "qwT": qwT,
                "qb": qkv_b,
                "qb16": qb16,
                "pwT": pwT,
                "pb": proj_b,
                "nw": norm_w,
                "nb": norm_b,
            }
        )
    return in_maps


def _run(inputs, trace=False, loop_n=None, ablate=()):
    nc = _get_program(loop_n, ablate)
    in_maps = _make_in_maps(inputs)
    res = run_bass_kernel_spmd(
        nc, in_maps, core_ids=list(range(NCORES)), trace=trace
    )
    y = np.stack([res.results[i]["y"] for i in range(NCORES)], axis=0)
    y = y.reshape(B, C, H, W)
    return y, res


def kernel(**inputs) -> np.ndarray:
    y, _ = _run(inputs, trace=False)
    return y


# revision 28
# speedup vs baseline: 1.0017x; 1.0017x over previous
"""AttentionBlock (GroupNorm + 4-head self-attention + proj + residual) on 8 TRN2 cores.

Data-parallel over batch: 16 batch elements -> 2 per NeuronCore; no collectives.
All matmul operands are fp16 (fp32 PSUM accumulation); stats/residual stay fp32.

Layout strategy (per core, per batch element; channels C=512, tokens N=1024):
  - GroupNorm: bn_stats per 128-channel chunk, then an XOR-butterfly partition
    all-reduce (stream_shuffle+add, 4 stages) within each 16-channel group --
    no DMA re-layout; per-channel scale/bias folded into one tensor_scalar
    pass that also casts to fp16.
  - QKV: lhsT = host-pretransposed qkv_w^T chunks; Q,K produced as [d, n]
    (channel-major), V produced directly transposed as V^T = h^T @ Wv^T [n, d];
    qkv bias via per-partition ACT bias (Q,K) and a K=1 ones-row matmul (V).
  - Attention per (head, query-half) "unit": S^T = K^T Q via lhsT=K-slice (no
    PE transposes anywhere); exp on ACT straight out of PSUM (softmax max-shift
    skipped: inputs are bounded N(0,1)-scale so exp cannot overflow);
    denominator = ones^T @ P on PE, interleaved with the O^T = V @ P matmuls of
    the previous unit to keep PE dense; normalization r = 1/denom is broadcast
    across partitions with a K=1 outer-product matmul and applied in the
    PSUM->SBUF copy of O^T. Tails are deferred two units so the cross-engine
    reciprocal chain never stalls PE.
  - proj: lhsT = proj_w^T chunks over O^T; residual + bias fused in one
    scalar_tensor_tensor. Output fp32.

Environment workaround: this walrus build encodes at most one semaphore wait
per instruction; _split_multiwait() moves excess waits onto injected
same-engine NoOps after TileContext scheduling.
"""

import math

import numpy as np

import concourse.bass as bass
import concourse.mybir as mybir
from concourse import tile
from concourse.bass_utils import run_bass_kernel_spmd

# problem constants (self-contained by contract)
B, C, H, W = 16, 512, 32, 32
N = H * W
HEADS, D = 4, 128
G = 32
EPS = 1e-5
SCALE = 1.0 / math.sqrt(D)
NCORES = 8
NB = B // NCORES  # batch elems per core
CK = C // 128     # channel chunks
NT = N // 128     # token tiles
F32 = mybir.dt.float32
F32R = mybir.dt.float32r
F16 = mybir.dt.float16
FX = mybir.ActivationFunctionType
ALU = mybir.AluOpType

WARM_MMS = 44  # PE warmup matmuls issued during the groupnorm prologue


# --- workaround: this walrus encodes at most ONE sync wait per instruction ---
_waitctr = [0]


def _split_multiwait(nc):
    for fn in nc.m.functions:
        for bb in fn.blocks:
            out = []
            changed = False
            for inst in bb.instructions:
                si = inst.sync_info
                if si is not None and len(si.on_wait) > 1:
                    waits = list(si.on_wait)
                    for wt in waits[:-1]:
                        _waitctr[0] += 1
                        nop = mybir.InstNoOp(
                            name=f"I-waitsplit-{_waitctr[0]}", ins=[], outs=[]
                        )
                        nop.engine = inst.engine
                        nop.sync_info = mybir.SyncInfo(on_wait=[wt], on_update=[])
                        out.append(nop)
                    inst.sync_info = mybir.SyncInfo(
                        on_wait=[waits[-1]], on_update=list(si.on_update)
                    )
                    changed = True
                out.append(inst)
            if changed:
                bb.instructions = out


def _build(loop_n=None, ablate=()):
    ab = set(ablate)
    nc = bass.Bass("TRN2", target_bir_lowering=False, debug=False)

    x_d = nc.dram_tensor("x", [NB, C, N], F32, kind="ExternalInput")
    qwT_d = nc.dram_tensor("qwT", [C, 3 * C], F16, kind="ExternalInput")
    qb_d = nc.dram_tensor("qb", [3 * C], F32, kind="ExternalInput")
    qb16_d = nc.dram_tensor("qb16", [3 * C], F16, kind="ExternalInput")
    pwT_d = nc.dram_tensor("pwT", [C, C], F16, kind="ExternalInput")
    pb_d = nc.dram_tensor("pb", [C], F32, kind="ExternalInput")
    nw_d = nc.dram_tensor("nw", [C], F32, kind="ExternalInput")
    nb_d = nc.dram_tensor("nb", [C], F32, kind="ExternalInput")
    y_d = nc.dram_tensor("y", [NB, C, N], F32, kind="ExternalOutput")

    with tile.TileContext(nc) as tc:
        import contextlib

        with contextlib.ExitStack() as ctx:
            wpool = ctx.enter_context(tc.tile_pool(name="wpool", bufs=1))
            xpool = ctx.enter_context(tc.tile_pool(name="xpool", bufs=6))
            x2pool = ctx.enter_context(tc.tile_pool(name="x2pool", bufs=3))
            spool = ctx.enter_context(tc.tile_pool(name="spool", bufs=2))
            stp = ctx.enter_context(tc.tile_pool(name="stp", bufs=5))
            hpool = ctx.enter_context(tc.tile_pool(name="hpool", bufs=8))
            qkpool = ctx.enter_context(tc.tile_pool(name="qkpool", bufs=12))
            vpool = ctx.enter_context(tc.tile_pool(name="vpool", bufs=12))
            ppool = ctx.enter_context(tc.tile_pool(name="ppool", bufs=20))
            opool = ctx.enter_context(tc.tile_pool(name="opool", bufs=6))
            ounpool = ctx.enter_context(tc.tile_pool(name="ounpool", bufs=10))
            ypool = ctx.enter_context(tc.tile_pool(name="ypool", bufs=3))
            ps_s = ctx.enter_context(tc.tile_pool(name="ps_s", bufs=2, space="PSUM"))
            ps_d = ctx.enter_context(tc.tile_pool(name="ps_d", bufs=4, space="PSUM"))
            ps_q = ctx.enter_context(tc.tile_pool(name="ps_q", bufs=2, space="PSUM"))

            # ---- constants & weights ----
            ones_col = wpool.tile([128, 1], F16, tag="ones_col")
            nc.gpsimd.memset(ones_col[:], 1.0)
            ones_row = wpool.tile([1, 128], F16, tag="ones_row")
            nc.gpsimd.memset(ones_row[:], 1.0)
            ones_w = wpool.tile([128, 512], F16, tag="ones_w")
            nc.gpsimd.memset(ones_w[:], 1.0)

            # PE warmup: overlaps the DMA/stats prologue so HAM reaches 2.4GHz
            # before the qkv matmuls start.
            warm_ps = ps_q.tile([128, 512], F32, tag="psq")
            for _ in range(0 if "no_warm" in ab else WARM_MMS):
                nc.tensor.matmul(
                    warm_ps[0:1, :], ones_col[:], ones_w[:], start=True, stop=True
                )

            wq = []
            for ck in range(CK):
                t = wpool.tile([128, 3 * C], F16, tag=f"wq{ck}")
                nc.sync.dma_start(
                    t[:], qwT_d.ap()[128 * ck : 128 * (ck + 1), :]
                )
                wq.append(t)
            wp = []
            for ck in range(CK):
                t = wpool.tile([128, C], F16, tag=f"wp{ck}")
                nc.sync.dma_start(
                    t[:], pwT_d.ap()[128 * ck : 128 * (ck + 1), :]
                )
                wp.append(t)
            qbv = []
            for t8 in range(8):
                t = wpool.tile([128, 1], F32, tag=f"qbv{t8}")
                nc.sync.dma_start(
                    t[:],
                    qb_d.ap()[128 * t8 : 128 * (t8 + 1)].rearrange(
                        "(p o) -> p o", o=1
                    ),
                )
                qbv.append(t)
            qbv_row = wpool.tile([1, C], F16, tag="qbv_row")
            nc.sync.dma_start(
                qbv_row[:],
                qb16_d.ap()[2 * C : 3 * C].rearrange("(o n) -> o n", o=1),
            )
            pbv = []
            for t4 in range(CK):
                t = wpool.tile([128, 1], F32, tag=f"pbv{t4}")
                nc.sync.dma_start(
                    t[:],
                    pb_d.ap()[128 * t4 : 128 * (t4 + 1)].rearrange(
                        "(p o) -> p o", o=1
                    ),
                )
                pbv.append(t)
            eps128 = wpool.tile([128, 1], F32, tag="eps")
            nc.gpsimd.memset(eps128[:], EPS)
            nwv, nbv = [], []
            for ck in range(CK):
                t = wpool.tile([128, 1], F32, tag=f"nwv{ck}")
                nc.sync.dma_start(
                    t[:],
                    nw_d.ap()[128 * ck : 128 * (ck + 1)].rearrange("(p o) -> p o", o=1),
                )
                nwv.append(t)
                t2 = wpool.tile([128, 1], F32, tag=f"nbv{ck}")
                nc.sync.dma_start(
                    t2[:],
                    nb_d.ap()[128 * ck : 128 * (ck + 1)].rearrange("(p o) -> p o", o=1),
                )
                nbv.append(t2)

            # ---- per-batch phase emitters ----
            def emit_norm(b):
                """GroupNorm via per-channel bn_stats + XOR-butterfly partition
                all-reduce (stream_shuffle) within each 16-channel group."""
                hs = []
                for ck in range(CK):
                    xt = xpool.tile([128, N], F32, tag="x", name=f"xt{b}_{ck}")
                    nc.sync.dma_start(
                        xt[:], x_d.ap()[b, 128 * ck : 128 * (ck + 1), :]
                    )
                    st = stp.tile([128, 12], F32, tag="st")
                    nc.vector.bn_stats(st[:, 0:6], xt[:, 0:512])
                    nc.vector.bn_stats(st[:, 6:12], xt[:, 512:1024])
                    str_ = st[:, :].rearrange("p (f t) -> p t f", t=3)
                    sum_m = stp.tile([128, 1], F32, tag="sum_m")
                    nc.vector.tensor_reduce(
                        sum_m[:], str_[:, 1, :], mybir.AxisListType.X, ALU.add
                    )
                    msq = stp.tile([128, 4], F32, tag="msq")
                    nc.vector.tensor_mul(msq[:], str_[:, 1, :], str_[:, 1, :])
                    sum_m2 = stp.tile([128, 1], F32, tag="sum_m2")
                    nc.vector.tensor_reduce(
                        sum_m2[:], msq[:], mybir.AxisListType.X, ALU.add
                    )
                    sum_cv = stp.tile([128, 1], F32, tag="sum_cv")
                    nc.vector.tensor_reduce(
                        sum_cv[:], str_[:, 2, :], mybir.AxisListType.X, ALU.add
                    )
                    # sg = [sum(x), sum(x^2)] per channel (512 elems each)
                    sg = stp.tile([128, 2], F32, tag="sg")
                    nc.vector.tensor_scalar(
                        sg[:, 0:1], sum_m[:], 256.0, None, ALU.mult
                    )
                    nc.vector.scalar_tensor_tensor(
                        sg[:, 1:2], sum_m2[:], 256.0, sum_cv[:], ALU.mult, ALU.add
                    )
                    for k in (1, 2, 4, 8):
                        tmp = stp.tile([128, 2], F32, tag="shuf")
                        nc.vector.stream_shuffle(
                            tmp[:], sg[:], [i ^ k for i in range(32)]
                        )
                        sg2 = stp.tile([128, 2], F32, tag="sg", name=f"sg{b}_{ck}_{k}")
                        nc.vector.tensor_add(sg2[:], sg[:], tmp[:])
                        sg = sg2
                    mean = stp.tile([128, 1], F32, tag="mean")
                    nc.vector.tensor_scalar(
                        mean[:], sg[:, 0:1], 1.0 / 16384.0, None, ALU.mult
                    )
                    e2 = stp.tile([128, 1], F32, tag="e2")
                    nc.vector.tensor_scalar(
                        e2[:], sg[:, 1:2], 1.0 / 16384.0, None, ALU.mult
                    )
                    msq2 = stp.tile([128, 1], F32, tag="msq2")
                    nc.vector.tensor_scalar(
                        msq2[:], mean[:], mean[:, 0:1], None, ALU.mult
                    )
                    var = stp.tile([128, 1], F32, tag="var")
                    nc.vector.tensor_sub(var[:], e2[:], msq2[:])
                    std = stp.tile([128, 1], F32, tag="std")
                    nc.scalar.activation(
                        std[:], var[:], FX.Sqrt, bias=eps128[:, 0:1]
                    )
                    rstd = stp.tile([128, 1], F32, tag="rstd")
                    nc.vector.reciprocal(rstd[:], std[:])
                    svec = stp.tile([128, 1], F32, tag="svec")
                    nc.vector.tensor_mul(svec[:], nwv[ck][:], rstd[:])
                    tms = stp.tile([128, 1], F32, tag="tms")
                    nc.vector.tensor_scalar(
                        tms[:], mean[:], svec[:, 0:1], None, ALU.mult
                    )
                    tvec = stp.tile([128, 1], F32, tag="tvec")
                    nc.vector.tensor_sub(tvec[:], nbv[ck][:], tms[:])
                    ht = hpool.tile([128, N], F16, tag="h")
                    if "no_norm" in ab:
                        nc.vector.tensor_copy(ht[:], xt[:])
                    else:
                        nc.vector.tensor_scalar(
                            ht[:], xt[:], svec[:, 0:1], tvec[:, 0:1],
                            ALU.mult, ALU.add,
                        )
                    hs.append(ht)
                return hs

            def emit_qkv(b, hs):
                """QK as [d,n] per head-slice tile; V^T as [n, 512]."""
                qk = []
                for t8 in range(8):
                    sb = qkpool.tile([128, N], F16, tag="qk")
                    for qh in range(2):
                        ps = ps_q.tile([128, 512], F32, tag="psq", name=f"qkps{b}_{t8}_{qh}")
                        for ck in range(CK):
                            nc.tensor.matmul(
                                ps[:],
                                wq[ck][:, 128 * t8 : 128 * (t8 + 1)],
                                hs[ck][:, 512 * qh : 512 * (qh + 1)],
                                start=(ck == 0),
                                stop=(ck == CK - 1),
                            )
                        nc.scalar.activation(
                            sb[:, 512 * qh : 512 * (qh + 1)], ps[:],
                            FX.Identity, bias=qbv[t8][:, 0:1],
                        )
                    qk.append(sb)
                vts = []
                for nt in range(NT):
                    ps = ps_q.tile([128, 512], F32, tag="psq", name=f"vps{b}_{nt}")
                    for ck in range(CK):
                        nc.tensor.matmul(
                            ps[:],
                            hs[ck][:, 128 * nt : 128 * (nt + 1)],
                            wq[ck][:, 2 * C : 3 * C],
                            start=(ck == 0),
                            stop=False,
                        )
                    nc.tensor.matmul(
                        ps[:], ones_row[:], qbv_row[:], start=False, stop=True
                    )
                    vt = vpool.tile([128, C], F16, tag="v")
                    nc.vector.tensor_copy(vt[:], ps[:])
                    vts.append(vt)
                return qk, vts

            # attention: units are (head, query-half); dn/O of unit u-1 are
            # interleaved with the S/exp stream of unit u to keep PE dense.
            def emit_unit(b, qk, vts, unit, prev):
                h, qh = unit
                q_sb, k_sb = qk[h], qk[HEADS + h]
                ptiles = []
                dn_ps = ot_ps = None
                if prev is not None:
                    if "no_dn" not in ab:
                        dn_ps = ps_d.tile([128, 512], F32, tag="psd")
                    ot_ps = ps_d.tile([128, 512], F32, tag="psd")
                for kt in range(NT):
                    if prev is not None:
                        ph, pptiles = prev[1][0], prev[2]
                        if "no_dn" not in ab:
                            nc.tensor.matmul(
                                dn_ps[0:1, :], ones_col[:], pptiles[kt][:],
                                start=(kt == 0), stop=(kt == NT - 1),
                            )
                        nc.tensor.matmul(
                            ot_ps[:], vts[kt][:, 128 * ph : 128 * (ph + 1)],
                            pptiles[kt][:],
                            start=(kt == 0), stop=(kt == NT - 1),
                        )
                    s_ps = ps_s.tile([128, 512], F32, tag="pss")
                    nc.tensor.matmul(
                        s_ps[:],
                        k_sb[:, 128 * kt : 128 * (kt + 1)],
                        q_sb[:, 512 * qh : 512 * (qh + 1)],
                        start=True, stop=True,
                    )
                    pt = ppool.tile([128, 512], F16, tag="p")
                    if "exp_dve" in ab:
                        nc.vector.tensor_copy(pt[:], s_ps[:])
                    else:
                        nc.scalar.activation(pt[:], s_ps[:], FX.Exp, scale=SCALE)
                    ptiles.append(pt)
                return ptiles, dn_ps, ot_ps

            def emit_unit_tail(b, prev, dn_ps, ot_ps, osb):
                """normalize: r = 1/denom; R = ones x r; o = O * R (from PSUM)."""
                h, qh = prev[1]
                if "no_dn" in ab:
                    nc.vector.tensor_copy(
                        osb[h][:, 512 * qh : 512 * (qh + 1)], ot_ps[:]
                    )
                    return
                r1 = spool.tile([1, 512], F32, tag="r1", bufs=4)
                nc.vector.reciprocal(r1[:], dn_ps[0:1, :])
                r1r = spool.tile([1, 512], F16, tag="r1r", bufs=4)
                nc.vector.tensor_copy(r1r[:], r1[:])
                R_ps = ps_q.tile([128, 512], F32, tag="psq")
                nc.tensor.matmul(R_ps[:], ones_row[:], r1r[:], start=True, stop=True)
                R_sb = spool.tile([128, 512], F32, tag="Rsb")
                nc.scalar.copy(R_sb[:], R_ps[:])
                nc.vector.tensor_mul(
                    osb[h][:, 512 * qh : 512 * (qh + 1)], ot_ps[:], R_sb[:]
                )

            def emit_attn(b, qk, vts, tail_ctx):
                osb = [
                    opool.tile([128, N], F16, tag="o", name=f"osb{b}_{i}")
                    for i in range(HEADS)
                ]
                units = [(h, qh) for h in range(HEADS) for qh in range(2)]
                prev = tail_ctx.get("prev")
                pending = tail_ctx.setdefault("pending", [])
                for u in units:
                    pt, dn_ps, ot_ps = emit_unit(b, qk, vts, u, prev)
                    if prev is not None:
                        pending.append((prev[0], prev, dn_ps, ot_ps, prev[3]))
                    while len(pending) > 1:
                        emit_unit_tail(*pending.pop(0))
                    prev = (b, u, pt, osb)
                tail_ctx["prev"] = prev
                return osb

            def emit_attn_flush(tail_ctx, vts_by_b):
                prev = tail_ctx.pop("prev", None)
                pending = tail_ctx.setdefault("pending", [])
                if prev is not None:
                    pb_, (ph, pqh), pptiles, posb = prev
                    dn_ps = None
                    if "no_dn" not in ab:
                        dn_ps = ps_d.tile([128, 512], F32, tag="psd")
                    ot_ps = ps_d.tile([128, 512], F32, tag="psd")
                    for kt in range(NT):
                        if "no_dn" not in ab:
                            nc.tensor.matmul(
                                dn_ps[0:1, :], ones_col[:], pptiles[kt][:],
                                start=(kt == 0), stop=(kt == NT - 1),
                            )
                        nc.tensor.matmul(
                            ot_ps[:], vts_by_b[pb_][kt][:, 128 * ph : 128 * (ph + 1)],
                            pptiles[kt][:],
                            start=(kt == 0), stop=(kt == NT - 1),
                        )
                    pending.append((pb_, prev, dn_ps, ot_ps, posb))
                while pending:
                    emit_unit_tail(*pending.pop(0))

            def emit_proj(b, osb):
                for t4 in range(CK):
                    xt2 = x2pool.tile([128, N], F32, tag="x2")
                    nc.sync.dma_start(
                        xt2[:], x_d.ap()[b, 128 * t4 : 128 * (t4 + 1), :]
                    )
                    yt = ypool.tile([128, N], F32, tag="y")
                    for qh in range(2):
                        ps = ps_q.tile([128, 512], F32, tag="psq", name=f"prps{b}_{t4}_{qh}")
                        for ck in range(CK):
                            nc.tensor.matmul(
                                ps[:],
                                wp[ck][:, 128 * t4 : 128 * (t4 + 1)],
                                osb[ck][:, 512 * qh : 512 * (qh + 1)],
                                start=(ck == 0),
                                stop=(ck == CK - 1),
                            )
                        nc.vector.scalar_tensor_tensor(
                            yt[:, 512 * qh : 512 * (qh + 1)], ps[:],
                            pbv[t4][:, 0:1], xt2[:, 512 * qh : 512 * (qh + 1)],
                            ALU.add, ALU.add,
                        )
                    nc.sync.dma_start(
                        y_d.ap()[b, 128 * t4 : 128 * (t4 + 1), :], yt[:]
                    )

            # ---- schedule ----
            def schedule():
                tail_ctx = {}
                vts_by_b = {}
                hs0 = emit_norm(0)
                qk0, vts0 = emit_qkv(0, hs0)
                vts_by_b[0] = vts0
                hs1 = emit_norm(1)
                osb0 = emit_attn(0, qk0, vts0, tail_ctx)
                qk1, vts1 = emit_qkv(1, hs1)
                vts_by_b[1] = vts1
                emit_attn_flush(tail_ctx, vts_by_b)
                emit_proj(0, osb0)
                osb1 = emit_attn(1, qk1, vts1, tail_ctx)
                emit_attn_flush(tail_ctx, vts_by_b)
                emit_proj(1, osb1)

            if loop_n is None:
                schedule()
            else:
                with tc.For_i(0, loop_n, 1):
                    schedule()

    _split_multiwait(nc)
    return nc


_CACHE = {}


def _get_program(loop_n=None, ablate=()):
    key = ("nc", loop_n, tuple(sorted(ablate)))
    if key not in _CACHE:
        _CACHE[key] = _build(loop_n, ablate)
    return _CACHE[key]


def _make_in_maps(inputs):
    x = np.ascontiguousarray(np.asarray(inputs["x"], dtype=np.float32))
    qkv_w = np.asarray(inputs["qkv_w"], dtype=np.float32)
    qkv_b = np.ascontiguousarray(np.asarray(inputs["qkv_b"], dtype=np.float32))
    proj_w = np.asarray(inputs["proj_w"], dtype=np.float32)
    proj_b = np.ascontiguousarray(np.asarray(inputs["proj_b"], dtype=np.float32))
    norm_w = np.ascontiguousarray(np.asarray(inputs["norm_w"], dtype=np.float32))
    norm_b = np.ascontiguousarray(np.asarray(inputs["norm_b"], dtype=np.float32))
    qwT = np.ascontiguousarray(qkv_w.T.astype(np.float16))
    pwT = np.ascontiguousarray(proj_w.T.astype(np.float16))
    qb16 = qkv_b.astype(np.float16)
    xs = x.reshape(NCORES, NB, C, N)
    in_maps = []
    for i in range(NCORES):
        in_maps.append(
            {
                "x": np.ascontiguousarray(xs[i]),
                "qwT": qwT,
                "qb": qkv_b,
                "qb16": qb16,
                "pwT": pwT,
                "pb": proj_b,
                "nw": norm_w,
                "nb": norm_b,
            }
        )
    return in_maps


def _run(inputs, trace=False, loop_n=None, ablate=()):
    nc = _get_program(loop_n, ablate)
    in_maps = _make_in_maps(inputs)
    res = run_bass_kernel_spmd(
        nc, in_maps, core_ids=list(range(NCORES)), trace=trace
    )
    y = np.stack([res.results[i]["y"] for i in range(NCORES)], axis=0)
    y = y.reshape(B, C, H, W)
    return y, res


def kernel(**inputs) -> np.ndarray:
    y, _ = _run(inputs, trace=False)
    return y
